# revision 22
# baseline (speedup 1.0000x reference)
"""BSMatchStar Trainium2 kernel (custom-DVE carry path; baseline 654us).

out = (a | (((a&b) +_brev b) ^ b)) -- bitstream MatchStar via a 2^29-bit
big-integer addition over per-byte bit-reversed operands.

Per core (1/8 contiguous slice, 8 tiles of [128, 2048] uint32 limbs),
processed in software-pipelined waves of W=2 tiles:
  Phase 1: X=brev(a&b), Y=brev(b) by SWAR (per level: A=(x<<s)&mL [ts 2x],
    B=(x>>s)&mR [ts 2x], y=A|B [tt 1x]); T=X+Y on GPSIMD (the only engine
    with an exact wrapping uint32 add); limb flags by integer compare:
    g01=(T<X) [tt is_lt], n01=(T!=~0) [ts not_equal]; then TWO custom-DVE
    ops (1 elem/cyc, vs 2.4 for stock scans) compute the row-local carry
    structure: CARRY_ENC m=(2*cumsum(n)+g+2)*n and MAX_SCAN
    r=cummax(m, init=2). parity(r) = running carry; r==2 = all-propagate.
  Stitch per wave: per-row (G,P)=(parity(r[-1]), r[-1]<3) bounce through
    DRAM to transpose into one [1, W*128] stream; one seeded mult/max scan
    produces every row's carry-in seed (decoupled lookback).
  Phase 2: seeded carries via 2 cheap ts ops (no second scan!):
    t=max(r, seed+2), ci[j]=t[j-1]&1; S'=T+ci (GPSIMD);
    out = (brev(S') ^ b) | a  (b stays SBUF-resident; a re-streamed;
    next wave's inputs prefetched as buffers free up).
  Cross-core carry resolved host-side (decoupled lookback; O(1) bytes of
  fixup per core boundary).

Engine discipline: GPSIMD and DVE run ~1:1 lockstep when overlapped
(shared SBUF pipe), so GPSIMD does only the two integer adds per tile;
everything else rides the DVE. Custom-DVE table ops (InstCustomDveAnt)
do the carry scan at 1 elem/cyc; the stock tensor_tensor_scan (2.4
cyc/elem) is not used at all.
"""
import sys
sys.path.insert(0, "/opt/trn_rl_repo")

import numpy as np

N_BYTES = 67_108_864
N_CORES = 8
P = 128
F = 2048
WORDS_PER_CORE = N_BYTES // 4 // N_CORES
T = WORDS_PER_CORE // (P * F)  # 8

_BREV = np.array([int(f"{i:08b}"[::-1], 2) for i in range(256)], dtype=np.uint8)

_cache = {}
_ops = {}

LVLS = [(4, 0xF0F0F0F0, 0x0F0F0F0F),
        (2, 0xCCCCCCCC, 0x33333333),
        (1, 0xAAAAAAAA, 0x55555555)]


def _register_custom_ops():
    """Register the carry-scan custom-DVE ops (idempotent)."""
    if _ops:
        return _ops
    import concourse.dve_ops as dve_ops
    from concourse.dve_spec import Spec, Src0, Src1, C0, C1, scan, lower
    from concourse.dve_spec import AluOp as SAluOp
    from concourse.dve_uop import DveOpSpec
    from concourse.dve_ops import DveOp

    def reg(name, spec, rd1):
        if name in dve_ops._SUB_OPCODE_FOR_NAME:
            return next(o for o in dve_ops.OPS if o.name == name)
        uops = lower(spec, ver="v3")
        row = dve_ops._CUSTOM_DVE_ROW_BASE + len(dve_ops.OPS)
        assert row < 0x20
        sha = DveOpSpec(name=name, opcode=row, uops=uops, rd1_en=rd1).sha("v3")
        op = DveOp(name, spec, subdim=False, uops_sha={"v3": sha})
        dve_ops.OPS.append(op)
        dve_ops._SUB_OPCODE_FOR_NAME[name] = row
        dve_ops.CUSTOM_DVE_SPECS[name] = spec
        return op

    def _ref_enc(in0, in1, c0, c1, c2):
        K = np.cumsum(in1.astype(np.float32), axis=-1)
        m = (2 * K + in0.astype(np.float32) + c1) * in1
        return m, m.max(axis=-1, keepdims=True)

    def _ref_mscan(in0, in1, c0, c1, c2):
        return np.maximum.accumulate(
            np.maximum(in0.astype(np.float32), c0), axis=-1)

    _K = scan(SAluOp.ADD, Src1)
    _m = (((_K + _K) + Src0) + C1) * Src1
    _ops["enc"] = reg(
        "CARRY_ENC",
        Spec(body=_m, accum=SAluOp.MAX, accum_init=C1, reference=_ref_enc),
        True)
    _ops["mscan"] = reg(
        "MAX_SCAN",
        Spec(body=scan(SAluOp.MAX, Src0, init=C0), reference=_ref_mscan),
        False)
    return _ops


def _build(n_tiles, f):
    import concourse.bacc as bacc
    import concourse.tile as tile
    import concourse.mybir as mybir
    import contextlib

    ops = _register_custom_ops()
    AOT = mybir.AluOpType
    dt = mybir.dt

    nc = bacc.Bacc("TRN2", target_bir_lowering=False, debug=False)

    d_a = nc.dram_tensor("a", [n_tiles, P, f], dt.uint32, kind="ExternalInput")
    d_b = nc.dram_tensor("b", [n_tiles, P, f], dt.uint32, kind="ExternalInput")
    d_o = nc.dram_tensor("o", [n_tiles, P, f], dt.uint32, kind="ExternalOutput")
    d_agg = nc.dram_tensor("agg", [1, 2], dt.float32, kind="ExternalOutput")
    d_gp = nc.dram_tensor("scr_gp", [2, n_tiles, P], dt.int8)
    d_seed = nc.dram_tensor("scr_seed", [1, n_tiles, P], dt.float32)

    def ts(out, in0, s1, s2, op0, op1=None):
        if op1 is not None:
            nc.vector.tensor_scalar(out, in0, s1, s2, op0, op1)
        else:
            nc.vector.tensor_scalar(out, in0, s1, s2, op0)

    def tt(out, a, b, op):
        nc.vector.tensor_tensor(out, a, b, op)

    def gp_add(out, a, b):
        nc.gpsimd.tensor_tensor(out, a, b, AOT.add)

    with tile.TileContext(nc) as tc, contextlib.ExitStack() as ctx:
        pool = ctx.enter_context(tc.tile_pool(name="sb", bufs=1))

        W = 2
        n_waves = n_tiles // W
        SLOTS = 2 * W  # two waves of resident state

        T_t = [pool.tile([P, f], dt.uint32, tag=f"T{s}", name=f"T{s}")
               for s in range(SLOTS)]
        m_t = [pool.tile([P, f], dt.uint16, tag=f"m{s}", name=f"m{s}")
               for s in range(SLOTS)]
        b_t = [pool.tile([P, f], dt.uint32, tag=f"b{s}", name=f"b{s}")
               for s in range(SLOTS)]

        def scratch(tag, t, dtype=dt.uint32, shape=None):
            return pool.tile(shape or [P, f], dtype, tag=f"{tag}_{t % 2}",
                             name=f"{tag}_{t % 2}")

        gpw = [pool.tile([P, 2, W], dt.int8, tag=f"gpw_{k % 2}",
                         name=f"gpw_{k % 2}") for k in range(2)]
        seeds_p = [pool.tile([P, 1], dt.float32, tag=f"seeds_{i}",
                             name=f"seeds_{i}") for i in range(2)]
        gt_p = [pool.tile([1, W * P], dt.int8, tag=f"gt_{i}", name=f"gt_{i}")
                for i in range(2)]
        a_in = [pool.tile([P, f], dt.uint32, tag=f"ain_{i}", name=f"ain_{i}")
                for i in range(2)]
        pt_p = [pool.tile([1, W * P], dt.int8, tag=f"pt_{i}", name=f"pt_{i}")
                for i in range(2)]
        g8 = pool.tile([P, f], dt.int8, tag="g8", name="g8")
        n8 = pool.tile([P, f], dt.int8, tag="n8", name="n8")
        amax = [pool.tile([P, 1], dt.float32, tag=f"am_{i}", name=f"am_{i}")
                for i in range(2)]
        amu = [pool.tile([P, 1], dt.uint16, tag=f"amu_{i}", name=f"amu_{i}")
               for i in range(2)]
        gc16 = [pool.tile([P, 1], dt.uint16, tag=f"gc_{i}", name=f"gc_{i}")
                for i in range(2)]
        sc2 = [pool.tile([P, 1], dt.float32, tag=f"sc2_{i}", name=f"sc2_{i}")
               for i in range(2)]

        live = {}

        def brev_chain(t, src, final):
            """3 swap levels; mids cycle through the 't0' scratch tag."""
            x = src
            for li, (sh, mL, mR) in enumerate(LVLS):
                A = scratch("A", t)
                B = scratch("Braw", t)
                y = final if li == 2 else scratch("t0", t)
                yield lambda x=x, sh=sh, mL=mL, A=A: ts(
                    A[:], x[:], sh, mL,
                    AOT.logical_shift_left, AOT.bitwise_and)
                yield lambda x=x, sh=sh, mR=mR, B=B: ts(
                    B[:], x[:], sh, mR,
                    AOT.logical_shift_right, AOT.bitwise_and)
                yield lambda y=y, B=B, A=A: tt(
                    y[:], A[:], B[:], AOT.bitwise_or)
                x = y

        # ---------------- phase 1 ------------------------------------------
        def phase1a_ops(t):
            s = t % SLOTS
            a_t = a_in[t % 2]
            if t < 2 * W:          # later waves are prefetched from phase 2
                yield lambda: nc.sync.dma_start(a_t[:], d_a[t])
                yield lambda: nc.sync.dma_start(b_t[s][:], d_b[t])

            t0 = scratch("t0", t)
            yield lambda: tt(t0[:], a_t[:], b_t[s][:], AOT.bitwise_and)

            X = scratch("xx", t)   # overwrites a_t after t0 consumed it
            live["X", t % 2] = X
            Y = pool.tile([P, f], dt.uint32, tag=f"ci_{t % 2}",
                          name=f"Y_{t % 2}")   # ci tag free during phase 1
            yield from brev_chain(t, t0, X)
            yield from brev_chain(t, b_t[s], Y)

            yield lambda: gp_add(T_t[s][:], X[:], Y[:])

        def phase1b_ops(t):
            s = t % SLOTS
            k = (t // W) % 2
            X = live["X", t % 2]
            am = amax[t % 2]
            yield lambda: tt(g8[:], T_t[s][:], X[:], AOT.is_lt)
            yield lambda: ts(n8[:], T_t[s][:], 0xFFFFFFFF, None,
                             AOT.not_equal)
            yield lambda: nc.vector._custom_dve(
                ops["enc"], out=m_t[s][:], accum_out=am[:],
                in0=g8[:], in1=n8[:], s1=2.0)

            au = amu[t % 2]
            yield lambda: nc.vector.tensor_copy(au[:], am[:])
            gc = gc16[t % 2]
            yield lambda: ts(gc[:], au[:], 1, None, AOT.bitwise_and)
            yield lambda: nc.vector.tensor_copy(gpw[k][:, 0, t % W:t % W + 1],
                                                gc[:])
            yield lambda: ts(gpw[k][:, 1, t % W:t % W + 1],
                             au[:], 3, None, AOT.is_lt)

        # ---------------- per-wave stitch ----------------------------------
        carry = pool.tile([1, 1], dt.float32, tag="carry_init")
        nc.vector.memset(carry[:], 0.0)
        pcore = pool.tile([1, 1], dt.float32, tag="pcore_init")
        nc.vector.memset(pcore[:], 1.0)

        def stitch_out(kw):
            k = kw % 2
            w0 = kw * W
            nc.sync.dma_start(d_gp[0, w0:w0 + W].rearrange("t p -> p t"),
                              gpw[k][:, 0, :])
            nc.sync.dma_start(d_gp[1, w0:w0 + W].rearrange("t p -> p t"),
                              gpw[k][:, 1, :])
            nc.sync.dma_start(
                gt_p[k][:],
                d_gp[0:1, w0:w0 + W].rearrange("one t p -> one (t p)"))
            nc.sync.dma_start(
                pt_p[k][:],
                d_gp[1:2, w0:w0 + W].rearrange("one t p -> one (t p)"))

        def stitch_in(kw):
            nonlocal carry, pcore
            k = kw % 2
            w0 = kw * W
            gt = gt_p[k]
            pt = pt_p[k]

            scw = pool.tile([1, W * P], dt.float32, tag=f"scw_{k}",
                            name=f"scw_{k}")
            nc.vector.tensor_tensor_scan(scw[:], pt[:], gt[:], carry[:],
                                         AOT.mult, AOT.max)
            seeds_row = pool.tile([1, W * P], dt.float32, tag=f"sr_{k}",
                                  name=f"sr_{k}")
            nc.vector.tensor_copy(seeds_row[0:1, 0:1], carry[:])
            nc.vector.tensor_copy(seeds_row[0:1, 1:], scw[0:1, :W * P - 1])
            nc.sync.dma_start(
                d_seed[0:1, w0:w0 + W, :].rearrange("one t p -> one (t p)"),
                seeds_row[:])

            for tt_ in range(w0, w0 + W):
                nc.sync.dma_start(
                    seeds_p[tt_ % 2][:],
                    d_seed[0, tt_:tt_ + 1, :].rearrange("one p -> p one"))

            ncarry = pool.tile([1, 1], dt.float32, tag=f"carry{kw}",
                               name=f"carry{kw}")
            nc.vector.tensor_copy(ncarry[:], scw[0:1, W * P - 1:W * P])
            carry = ncarry
            ptile = pool.tile([1, 1], dt.float32, tag=f"ptile{kw}",
                              name=f"ptile{kw}")
            nc.vector.tensor_reduce(ptile[:], pt[:], mybir.AxisListType.X,
                                    AOT.min)
            npcore = pool.tile([1, 1], dt.float32, tag=f"pcore{kw}",
                               name=f"pcore{kw}")
            nc.vector.tensor_tensor(npcore[:], pcore[:], ptile[:], AOT.min)
            pcore = npcore

        # ---------------- phase 2 ------------------------------------------
        def phase2a_ops(t):
            s = t % SLOTS
            seeds = seeds_p[t % 2]

            sc = sc2[t % 2]
            yield lambda: ts(sc[:], seeds[:], 2.0, None, AOT.add)
            r32 = scratch("r32", t)
            yield lambda: nc.vector._custom_dve(
                ops["mscan"], out=r32[:], in0=m_t[s][:], s0=sc[:])
            ci = scratch("ci", t)
            yield lambda: ts(ci[:, 1:f], r32[:, 0:f - 1], 1, None,
                             AOT.bitwise_and)
            yield lambda: nc.vector.tensor_copy(ci[:, 0:1], seeds[:])

            sp = scratch("t0", t)
            live["sp", t % 2] = sp
            yield lambda: gp_add(sp[:], T_t[s][:], ci[:])

            a2 = a_in[t % 2]
            yield lambda: nc.sync.dma_start(a2[:], d_a[t])

        def phase2b_ops(t):
            s = t % SLOTS
            sp = live["sp", t % 2]
            a2 = a_in[t % 2]
            wb = scratch("ci", t)
            yield from brev_chain(t, sp, wb)

            t1 = scratch("A", t)
            yield lambda: tt(t1[:], wb[:], b_t[s][:], AOT.bitwise_xor)
            if t + 2 * W < n_tiles:    # b slot free: prefetch wave kw+2
                yield lambda: nc.sync.dma_start(b_t[s][:], d_b[t + 2 * W])
            o_t = scratch("t0", t)
            yield lambda: tt(o_t[:], t1[:], a2[:], AOT.bitwise_or)
            if t + 2 * W < n_tiles:    # a buffer free: prefetch wave kw+2
                yield lambda: nc.sync.dma_start(a_in[t % 2][:],
                                                d_a[t + 2 * W])
            yield lambda: nc.sync.dma_start(d_o[t], o_t[:])

        # ---------------- pipelined emission -------------------------------
        def emit_pairs(gen_fn, kw, skew=7):
            # Skewed pairwise interleave: tile t runs `skew` ops ahead of
            # t+1 so their GPSIMD adds don't align (DVE fills the gap).
            for t in range(kw * W, (kw + 1) * W, 2):
                gens = [gen_fn(t), gen_fn(t + 1)]
                done = [False, False]
                for _ in range(skew):
                    try:
                        next(gens[0])()
                    except StopIteration:
                        done[0] = True
                        break
                while not all(done):
                    for i, g in enumerate(gens):
                        if not done[i]:
                            try:
                                next(g)()
                            except StopIteration:
                                done[i] = True

        def emit_phase1(kw, skew=7):
            emit_pairs(phase1a_ops, kw, skew=skew)
            # sequential: g8/n8/m16 are single-buffered (interleaving the
            # pair would put a WAR wait on a later same-queue instruction)
            emit_pairs(phase1b_ops, kw, skew=99)

        emit_phase1(0)
        stitch_out(0)
        emit_phase1(1)
        stitch_out(1)
        stitch_in(0)
        for kw in range(n_waves):
            emit_pairs(phase2a_ops, kw, skew=3)
            emit_pairs(phase2b_ops, kw, skew=2)
            if kw + 1 < n_waves:
                stitch_in(kw + 1)
            if kw + 2 < n_waves:
                emit_phase1(kw + 2)
                stitch_out(kw + 2)

        agg = pool.tile([1, 2], dt.float32, tag="agg")
        nc.vector.tensor_copy(agg[0:1, 0:1], carry[:])
        nc.vector.tensor_copy(agg[0:1, 1:2], pcore[:])
        nc.sync.dma_start(d_agg[:], agg[:])

    nc.compile()
    return nc


def _get_nc(n_tiles, f):
    key = (n_tiles, f)
    if key not in _cache:
        _cache[key] = _build(n_tiles, f)
    return _cache[key]


def run_sharded(a_u8, b_u8, n_cores=N_CORES, f=F, **spmd_kwargs):
    from concourse import bass_utils

    n = a_u8.size
    words = n // 4
    wpc = words // n_cores
    n_tiles = wpc // (P * f)
    assert n_tiles * P * f == wpc, (n, n_cores, f)

    a32 = a_u8.view(np.uint32).reshape(n_cores, n_tiles, P, f)
    b32 = b_u8.view(np.uint32).reshape(n_cores, n_tiles, P, f)

    nc = _get_nc(n_tiles, f)
    in_maps = [{"a": np.ascontiguousarray(a32[c]),
                "b": np.ascontiguousarray(b32[c])}
               for c in range(n_cores)]
    res = bass_utils.run_bass_kernel_spmd(nc, in_maps,
                                          core_ids=list(range(n_cores)),
                                          **spmd_kwargs)
    outs = [r["o"] for r in res.results]
    aggs = [(float(r["agg"][0, 0]), float(r["agg"][0, 1]))
            for r in res.results]
    out = np.concatenate([o.reshape(-1).view(np.uint8) for o in outs])
    return out, aggs, res


def _fixup_boundaries(out, a_u8, b_u8, aggs, n_cores):
    """Resolve the cross-core carry (decoupled lookback, host side)."""
    n = out.size
    csize = n // n_cores
    c_in = 0
    for c in range(n_cores):
        g_c = 1 if aggs[c][0] != 0.0 else 0
        p_c = 1 if aggs[c][1] != 0.0 else 0
        if c_in == 1:
            st = c * csize
            en = st + csize
            i = st
            done = False
            while i < en and not done:
                j = min(i + 65536, en)
                aa = a_u8[i:j]
                bb = b_u8[i:j]
                raw = _BREV[aa & bb].astype(np.int32) + _BREV[bb].astype(np.int32)
                prop = raw == 255
                if prop.all():
                    out[i:j] = aa | bb
                    i = j
                    continue
                k = int(np.argmin(prop))
                out[i:i + k] = aa[:k] | bb[:k]
                idx = i + k
                new_s = (int(raw[k]) + 1) & 0xFF
                out[idx] = (int(_BREV[new_s]) ^ int(b_u8[idx])) | int(a_u8[idx])
                done = True
        c_in = g_c | (p_c & c_in)
    return out


def kernel(a, b):
    assert a.dtype == np.uint8 and b.dtype == np.uint8 and a.size == N_BYTES
    out, aggs, _ = run_sharded(a, b)
    out = _fixup_boundaries(out, a, b, aggs, N_CORES)
    return out


# revision 24
# speedup vs baseline: 1.0343x; 1.0343x over previous
"""BSMatchStar Trainium2 kernel (custom-DVE carry path; baseline 654us).

out = (a | (((a&b) +_brev b) ^ b)) -- bitstream MatchStar via a 2^29-bit
big-integer addition over per-byte bit-reversed operands.

Per core (1/8 contiguous slice, 8 tiles of [128, 2048] uint32 limbs),
processed in software-pipelined waves of W=2 tiles:
  Phase 1: X=brev(a&b), Y=brev(b) by SWAR (per level: A=(x<<s)&mL [ts 2x],
    B=(x>>s)&mR [ts 2x], y=A|B [tt 1x]); T=X+Y on GPSIMD (the only engine
    with an exact wrapping uint32 add); limb flags by integer compare:
    g01=(T<X) [tt is_lt], n01=(T!=~0) [ts not_equal]; then TWO custom-DVE
    ops (1 elem/cyc, vs 2.4 for stock scans) compute the row-local carry
    structure: CARRY_ENC m=(2*cumsum(n)+g+2)*n and MAX_SCAN
    r=cummax(m, init=2). parity(r) = running carry; r==2 = all-propagate.
  Stitch per wave: per-row (G,P)=(parity(r[-1]), r[-1]<3) bounce through
    DRAM to transpose into one [1, W*128] stream; one seeded mult/max scan
    produces every row's carry-in seed (decoupled lookback).
  Phase 2: seeded carries via 2 cheap ts ops (no second scan!):
    t=max(r, seed+2), ci[j]=t[j-1]&1; S'=T+ci (GPSIMD);
    out = (brev(S') ^ b) | a  (b stays SBUF-resident; a re-streamed;
    next wave's inputs prefetched as buffers free up).
  Cross-core carry resolved host-side (decoupled lookback; O(1) bytes of
  fixup per core boundary).

Engine discipline: GPSIMD and DVE run ~1:1 lockstep when overlapped
(shared SBUF pipe), so GPSIMD does only the two integer adds per tile;
everything else rides the DVE. Custom-DVE table ops (InstCustomDveAnt)
do the carry scan at 1 elem/cyc; the stock tensor_tensor_scan (2.4
cyc/elem) is not used at all.
"""
import sys
sys.path.insert(0, "/opt/trn_rl_repo")

import numpy as np

N_BYTES = 67_108_864
N_CORES = 8
P = 128
F = 2048
WORDS_PER_CORE = N_BYTES // 4 // N_CORES
T = WORDS_PER_CORE // (P * F)  # 8

_BREV = np.array([int(f"{i:08b}"[::-1], 2) for i in range(256)], dtype=np.uint8)

_cache = {}
_ops = {}

LVLS = [(4, 0xF0F0F0F0, 0x0F0F0F0F),
        (2, 0xCCCCCCCC, 0x33333333),
        (1, 0xAAAAAAAA, 0x55555555)]


def _register_custom_ops():
    """Register the carry-scan custom-DVE ops (idempotent)."""
    if _ops:
        return _ops
    import concourse.dve_ops as dve_ops
    from concourse.dve_spec import Spec, Src0, Src1, C0, C1, scan, lower
    from concourse.dve_spec import AluOp as SAluOp
    from concourse.dve_uop import DveOpSpec
    from concourse.dve_ops import DveOp

    def reg(name, spec, rd1):
        if name in dve_ops._SUB_OPCODE_FOR_NAME:
            return next(o for o in dve_ops.OPS if o.name == name)
        uops = lower(spec, ver="v3")
        row = dve_ops._CUSTOM_DVE_ROW_BASE + len(dve_ops.OPS)
        assert row < 0x20
        sha = DveOpSpec(name=name, opcode=row, uops=uops, rd1_en=rd1).sha("v3")
        op = DveOp(name, spec, subdim=False, uops_sha={"v3": sha})
        dve_ops.OPS.append(op)
        dve_ops._SUB_OPCODE_FOR_NAME[name] = row
        dve_ops.CUSTOM_DVE_SPECS[name] = spec
        return op

    def _ref_enc(in0, in1, c0, c1, c2):
        K = np.cumsum(in1.astype(np.float32), axis=-1)
        m = (2 * K + in0.astype(np.float32) + c1) * in1
        return m, m.max(axis=-1, keepdims=True)

    def _ref_mscan(in0, in1, c0, c1, c2):
        return np.maximum.accumulate(
            np.maximum(in0.astype(np.float32), c0), axis=-1)

    _K = scan(SAluOp.ADD, Src1)
    _m = (((_K + _K) + Src0) + C1) * Src1
    _ops["enc"] = reg(
        "CARRY_ENC",
        Spec(body=_m, accum=SAluOp.MAX, accum_init=C1, reference=_ref_enc),
        True)
    _ops["mscan"] = reg(
        "MAX_SCAN",
        Spec(body=scan(SAluOp.MAX, Src0, init=C0), reference=_ref_mscan),
        False)
    return _ops


def _build(n_tiles, f):
    import concourse.bacc as bacc
    import concourse.tile as tile
    import concourse.mybir as mybir
    import contextlib

    ops = _register_custom_ops()
    AOT = mybir.AluOpType
    dt = mybir.dt

    nc = bacc.Bacc("TRN2", target_bir_lowering=False, debug=False)

    d_a = nc.dram_tensor("a", [n_tiles, P, f], dt.uint32, kind="ExternalInput")
    d_b = nc.dram_tensor("b", [n_tiles, P, f], dt.uint32, kind="ExternalInput")
    d_o = nc.dram_tensor("o", [n_tiles, P, f], dt.uint32, kind="ExternalOutput")
    d_agg = nc.dram_tensor("agg", [1, 2], dt.float32, kind="ExternalOutput")
    d_gp = nc.dram_tensor("scr_gp", [2, n_tiles, P], dt.int8)
    d_seed = nc.dram_tensor("scr_seed", [1, n_tiles, P], dt.float32)

    def ts(out, in0, s1, s2, op0, op1=None):
        if op1 is not None:
            nc.vector.tensor_scalar(out, in0, s1, s2, op0, op1)
        else:
            nc.vector.tensor_scalar(out, in0, s1, s2, op0)

    def tt(out, a, b, op):
        nc.vector.tensor_tensor(out, a, b, op)

    def gp_add(out, a, b):
        nc.gpsimd.tensor_tensor(out, a, b, AOT.add)

    with tile.TileContext(nc) as tc, contextlib.ExitStack() as ctx:
        pool = ctx.enter_context(tc.tile_pool(name="sb", bufs=1))

        W = 2
        n_waves = n_tiles // W
        SLOTS = 2 * W  # two waves of resident state

        T_t = [pool.tile([P, f], dt.uint32, tag=f"T{s}", name=f"T{s}")
               for s in range(SLOTS)]
        m_t = [pool.tile([P, f], dt.uint16, tag=f"m{s}", name=f"m{s}")
               for s in range(SLOTS)]
        b_t = [pool.tile([P, f], dt.uint32, tag=f"b{s}", name=f"b{s}")
               for s in range(SLOTS)]

        def scratch(tag, t, dtype=dt.uint32, shape=None):
            return pool.tile(shape or [P, f], dtype, tag=f"{tag}_{t % 2}",
                             name=f"{tag}_{t % 2}")

        gpw = [pool.tile([P, 2, W], dt.int8, tag=f"gpw_{k % 2}",
                         name=f"gpw_{k % 2}") for k in range(2)]
        seeds_p = [pool.tile([P, 1], dt.float32, tag=f"seeds_{i}",
                             name=f"seeds_{i}") for i in range(2)]
        gt_p = [pool.tile([1, W * P], dt.int8, tag=f"gt_{i}", name=f"gt_{i}")
                for i in range(2)]
        a_in = [pool.tile([P, f], dt.uint32, tag=f"ain_{i}", name=f"ain_{i}")
                for i in range(2)]
        pt_p = [pool.tile([1, W * P], dt.int8, tag=f"pt_{i}", name=f"pt_{i}")
                for i in range(2)]
        g8 = pool.tile([P, f], dt.int8, tag="g8", name="g8")
        n8 = pool.tile([P, f], dt.int8, tag="n8", name="n8")
        amax = [pool.tile([P, 1], dt.float32, tag=f"am_{i}", name=f"am_{i}")
                for i in range(2)]
        amu = [pool.tile([P, 1], dt.uint16, tag=f"amu_{i}", name=f"amu_{i}")
               for i in range(2)]
        gc16 = [pool.tile([P, 1], dt.uint16, tag=f"gc_{i}", name=f"gc_{i}")
                for i in range(2)]
        sc2 = [pool.tile([P, 1], dt.float32, tag=f"sc2_{i}", name=f"sc2_{i}")
               for i in range(2)]

        live = {}

        def brev_chain(t, src, final):
            """3 swap levels; mids cycle through the 't0' scratch tag."""
            x = src
            for li, (sh, mL, mR) in enumerate(LVLS):
                A = scratch("A", t)
                B = scratch("Braw", t)
                y = final if li == 2 else scratch("t0", t)
                yield lambda x=x, sh=sh, mL=mL, A=A: ts(
                    A[:], x[:], sh, mL,
                    AOT.logical_shift_left, AOT.bitwise_and)
                yield lambda x=x, sh=sh, mR=mR, B=B: ts(
                    B[:], x[:], sh, mR,
                    AOT.logical_shift_right, AOT.bitwise_and)
                yield lambda y=y, B=B, A=A: tt(
                    y[:], A[:], B[:], AOT.bitwise_or)
                x = y

        # ---------------- phase 1 ------------------------------------------
        def phase1a_ops(t):
            s = t % SLOTS
            a_t = a_in[t % 2]
            if t < 2 * W:          # later waves are prefetched from phase 2
                yield lambda: nc.sync.dma_start(a_t[:], d_a[t])
                yield lambda: nc.sync.dma_start(b_t[s][:], d_b[t])

            t0 = scratch("t0", t)
            yield lambda: tt(t0[:], a_t[:], b_t[s][:], AOT.bitwise_and)

            X = scratch("xx", t)   # overwrites a_t after t0 consumed it
            live["X", t % 2] = X
            Y = pool.tile([P, f], dt.uint32, tag=f"ci_{t % 2}",
                          name=f"Y_{t % 2}")   # ci tag free during phase 1
            yield from brev_chain(t, t0, X)
            yield from brev_chain(t, b_t[s], Y)

            yield lambda: gp_add(T_t[s][:], X[:], Y[:])

        def phase1b_ops(t):
            s = t % SLOTS
            k = (t // W) % 2
            X = live["X", t % 2]
            am = amax[t % 2]
            yield lambda: tt(g8[:], T_t[s][:], X[:], AOT.is_lt)
            yield lambda: ts(n8[:], T_t[s][:], 0xFFFFFFFF, None,
                             AOT.not_equal)
            yield lambda: nc.vector._custom_dve(
                ops["enc"], out=m_t[s][:], accum_out=am[:],
                in0=g8[:], in1=n8[:], s1=2.0)

            au = amu[t % 2]
            yield lambda: nc.vector.tensor_copy(au[:], am[:])
            gc = gc16[t % 2]
            yield lambda: ts(gc[:], au[:], 1, None, AOT.bitwise_and)
            yield lambda: nc.vector.tensor_copy(gpw[k][:, 0, t % W:t % W + 1],
                                                gc[:])
            yield lambda: ts(gpw[k][:, 1, t % W:t % W + 1],
                             au[:], 3, None, AOT.is_lt)

        # ---------------- per-wave stitch ----------------------------------
        carry = pool.tile([1, 1], dt.float32, tag="carry_init")
        nc.vector.memset(carry[:], 0.0)
        pcore = pool.tile([1, 1], dt.float32, tag="pcore_init")
        nc.vector.memset(pcore[:], 1.0)

        def stitch_out(kw):
            k = kw % 2
            w0 = kw * W
            nc.sync.dma_start(d_gp[0, w0:w0 + W].rearrange("t p -> p t"),
                              gpw[k][:, 0, :])
            nc.sync.dma_start(d_gp[1, w0:w0 + W].rearrange("t p -> p t"),
                              gpw[k][:, 1, :])
            nc.sync.dma_start(
                gt_p[k][:],
                d_gp[0:1, w0:w0 + W].rearrange("one t p -> one (t p)"))
            nc.sync.dma_start(
                pt_p[k][:],
                d_gp[1:2, w0:w0 + W].rearrange("one t p -> one (t p)"))

        def stitch_in(kw):
            nonlocal carry, pcore
            k = kw % 2
            w0 = kw * W
            gt = gt_p[k]
            pt = pt_p[k]

            scw = pool.tile([1, W * P], dt.float32, tag=f"scw_{k}",
                            name=f"scw_{k}")
            nc.vector.tensor_tensor_scan(scw[:], pt[:], gt[:], carry[:],
                                         AOT.mult, AOT.max)
            seeds_row = pool.tile([1, W * P], dt.float32, tag=f"sr_{k}",
                                  name=f"sr_{k}")
            nc.vector.tensor_copy(seeds_row[0:1, 0:1], carry[:])
            nc.vector.tensor_copy(seeds_row[0:1, 1:], scw[0:1, :W * P - 1])
            nc.sync.dma_start(
                d_seed[0:1, w0:w0 + W, :].rearrange("one t p -> one (t p)"),
                seeds_row[:])

            for tt_ in range(w0, w0 + W):
                nc.sync.dma_start(
                    seeds_p[tt_ % 2][:],
                    d_seed[0, tt_:tt_ + 1, :].rearrange("one p -> p one"))

            ncarry = pool.tile([1, 1], dt.float32, tag=f"carry{kw}",
                               name=f"carry{kw}")
            nc.vector.tensor_copy(ncarry[:], scw[0:1, W * P - 1:W * P])
            carry = ncarry
            ptile = pool.tile([1, 1], dt.float32, tag=f"ptile{kw}",
                              name=f"ptile{kw}")
            nc.vector.tensor_reduce(ptile[:], pt[:], mybir.AxisListType.X,
                                    AOT.min)
            npcore = pool.tile([1, 1], dt.float32, tag=f"pcore{kw}",
                               name=f"pcore{kw}")
            nc.vector.tensor_tensor(npcore[:], pcore[:], ptile[:], AOT.min)
            pcore = npcore

        # ---------------- phase 2 ------------------------------------------
        def phase2a_ops(t):
            s = t % SLOTS
            seeds = seeds_p[t % 2]

            sc = sc2[t % 2]
            yield lambda: nc.scalar.activation(
                sc[:], seeds[:], mybir.ActivationFunctionType.Copy, bias=2.0)
            r32 = scratch("r32", t)
            yield lambda: nc.vector._custom_dve(
                ops["mscan"], out=r32[:], in0=m_t[s][:], s0=sc[:])
            ci = scratch("ci", t)
            yield lambda: ts(ci[:, 1:f], r32[:, 0:f - 1], 1, None,
                             AOT.bitwise_and)
            yield lambda: nc.scalar.copy(ci[:, 0:1], seeds[:])

            sp = scratch("t0", t)
            live["sp", t % 2] = sp
            yield lambda: gp_add(sp[:], T_t[s][:], ci[:])

            a2 = a_in[t % 2]
            yield lambda: nc.sync.dma_start(a2[:], d_a[t])

        def phase2b_ops(t):
            s = t % SLOTS
            sp = live["sp", t % 2]
            a2 = a_in[t % 2]
            wb = scratch("ci", t)
            yield from brev_chain(t, sp, wb)

            t1 = scratch("A", t)
            yield lambda: tt(t1[:], wb[:], b_t[s][:], AOT.bitwise_xor)
            if t + 2 * W < n_tiles:    # b slot free: prefetch wave kw+2
                yield lambda: nc.sync.dma_start(b_t[s][:], d_b[t + 2 * W])
            o_t = scratch("t0", t)
            yield lambda: tt(o_t[:], t1[:], a2[:], AOT.bitwise_or)
            if t + 2 * W < n_tiles:    # a buffer free: prefetch wave kw+2
                yield lambda: nc.sync.dma_start(a_in[t % 2][:],
                                                d_a[t + 2 * W])
            yield lambda: nc.sync.dma_start(d_o[t], o_t[:])

        # ---------------- pipelined emission -------------------------------
        def emit_pairs(gen_fn, kw, skew=7):
            # Skewed pairwise interleave: tile t runs `skew` ops ahead of
            # t+1 so their GPSIMD adds don't align (DVE fills the gap).
            for t in range(kw * W, (kw + 1) * W, 2):
                gens = [gen_fn(t), gen_fn(t + 1)]
                done = [False, False]
                for _ in range(skew):
                    try:
                        next(gens[0])()
                    except StopIteration:
                        done[0] = True
                        break
                while not all(done):
                    for i, g in enumerate(gens):
                        if not done[i]:
                            try:
                                next(g)()
                            except StopIteration:
                                done[i] = True

        def emit_phase1(kw, skew=7):
            emit_pairs(phase1a_ops, kw, skew=skew)
            # sequential: g8/n8/m16 are single-buffered (interleaving the
            # pair would put a WAR wait on a later same-queue instruction)
            emit_pairs(phase1b_ops, kw, skew=99)

        emit_phase1(0)
        stitch_out(0)
        emit_phase1(1)
        stitch_out(1)
        stitch_in(0)
        for kw in range(n_waves):
            emit_pairs(phase2a_ops, kw, skew=3)
            emit_pairs(phase2b_ops, kw, skew=2)
            if kw + 1 < n_waves:
                stitch_in(kw + 1)
            if kw + 2 < n_waves:
                emit_phase1(kw + 2)
                stitch_out(kw + 2)

        agg = pool.tile([1, 2], dt.float32, tag="agg")
        nc.vector.tensor_copy(agg[0:1, 0:1], carry[:])
        nc.vector.tensor_copy(agg[0:1, 1:2], pcore[:])
        nc.sync.dma_start(d_agg[:], agg[:])

    nc.compile()
    return nc


def _get_nc(n_tiles, f):
    key = (n_tiles, f)
    if key not in _cache:
        _cache[key] = _build(n_tiles, f)
    return _cache[key]


def run_sharded(a_u8, b_u8, n_cores=N_CORES, f=F, **spmd_kwargs):
    from concourse import bass_utils

    n = a_u8.size
    words = n // 4
    wpc = words // n_cores
    n_tiles = wpc // (P * f)
    assert n_tiles * P * f == wpc, (n, n_cores, f)

    a32 = a_u8.view(np.uint32).reshape(n_cores, n_tiles, P, f)
    b32 = b_u8.view(np.uint32).reshape(n_cores, n_tiles, P, f)

    nc = _get_nc(n_tiles, f)
    in_maps = [{"a": np.ascontiguousarray(a32[c]),
                "b": np.ascontiguousarray(b32[c])}
               for c in range(n_cores)]
    res = bass_utils.run_bass_kernel_spmd(nc, in_maps,
                                          core_ids=list(range(n_cores)),
                                          **spmd_kwargs)
    outs = [r["o"] for r in res.results]
    aggs = [(float(r["agg"][0, 0]), float(r["agg"][0, 1]))
            for r in res.results]
    out = np.concatenate([o.reshape(-1).view(np.uint8) for o in outs])
    return out, aggs, res


def _fixup_boundaries(out, a_u8, b_u8, aggs, n_cores):
    """Resolve the cross-core carry (decoupled lookback, host side)."""
    n = out.size
    csize = n // n_cores
    c_in = 0
    for c in range(n_cores):
        g_c = 1 if aggs[c][0] != 0.0 else 0
        p_c = 1 if aggs[c][1] != 0.0 else 0
        if c_in == 1:
            st = c * csize
            en = st + csize
            i = st
            done = False
            while i < en and not done:
                j = min(i + 65536, en)
                aa = a_u8[i:j]
                bb = b_u8[i:j]
                raw = _BREV[aa & bb].astype(np.int32) + _BREV[bb].astype(np.int32)
                prop = raw == 255
                if prop.all():
                    out[i:j] = aa | bb
                    i = j
                    continue
                k = int(np.argmin(prop))
                out[i:i + k] = aa[:k] | bb[:k]
                idx = i + k
                new_s = (int(raw[k]) + 1) & 0xFF
                out[idx] = (int(_BREV[new_s]) ^ int(b_u8[idx])) | int(a_u8[idx])
                done = True
        c_in = g_c | (p_c & c_in)
    return out


def kernel(a, b):
    assert a.dtype == np.uint8 and b.dtype == np.uint8 and a.size == N_BYTES
    out, aggs, _ = run_sharded(a, b)
    out = _fixup_boundaries(out, a, b, aggs, N_CORES)
    return out
